# revision 21
# baseline (speedup 1.0000x reference)
"""Trainium2 Bass kernel: causal self-attention with GQA + RoPE + sliding window.

Model (hardcoded from the problem spec):
  D_MODEL=2048, N_HEADS=16 (head_dim 128), N_KV_HEADS=4, T=2048, B=2,
  SLIDING_WINDOW=512, THETA=10000.

Sharding: 8 cores = batch(2) x kv-groups(4). Core (b, g) handles batch b and
query heads 4g..4g+3 with kv head g (Wqkv column-sharded). Output projection
is row-sharded (rows 512g..512g+512); the 4 partial products per batch are
summed on the host.

On-chip layout is feature-major ("transposed"): x is fed pre-transposed
(host-side) as xT [d_model, T], the QKV projection produces qkv^T
[d_out, tok], attention runs on S^T = K@Q^T tiles [k, q] so softmax
normalization uses a ones-vector matmul for the partition-dim sum, and the
PV product directly yields O^T [dv, q] which is the natural lhsT for the
output projection. All matmuls run in float32r (TF32-like, full PE rate at
moving-dim >= 256).
"""

import math

import numpy as np

try:
    import concourse.bass as bass
except ImportError:  # pragma: no cover - environment fallback
    import sys

    sys.path.insert(0, "/opt/trn_rl_repo")
    import concourse.bass as bass

import concourse.mybir as mybir
import concourse.tile as tile
from concourse import bacc
from concourse.bass_utils import run_bass_kernel_spmd

D_MODEL = 2048
N_HEADS = 16
N_KV_HEADS = 4
HEAD_DIM = 128
KV_DIM = N_KV_HEADS * HEAD_DIM  # 512
T = 2048
B = 2
SW = 512
THETA = 10000.0

P = 128
SB = 512                 # token super-block
N_SB = T // SB           # 4
KC = D_MODEL // P        # 16 contraction chunks
QH = 4                   # query heads per core
DOUT = QH * HEAD_DIM + 2 * HEAD_DIM  # 768 sharded qkv out dim
MQK = DOUT // P          # 6 dout chunks (0..3 Q, 4 K, 5 V)
SCALE = 1.0 / math.sqrt(HEAD_DIM)

F32 = mybir.dt.float32
F32R = mybir.dt.float32r

_CACHE = {}


def _build_program():
    nc = bacc.Bacc("TRN2", target_bir_lowering=False, debug=False, num_devices=8)

    xT = nc.dram_tensor("xT", [D_MODEL, T], F32R, kind="ExternalInput").ap()
    wqkv = nc.dram_tensor("wqkv", [D_MODEL, DOUT], F32R, kind="ExternalInput").ap()
    wout = nc.dram_tensor("wout", [QH * HEAD_DIM, D_MODEL], F32R, kind="ExternalInput").ap()
    cosT = nc.dram_tensor("cosT", [P, T], F32, kind="ExternalInput").ap()
    sinS = nc.dram_tensor("sinS", [P, T], F32, kind="ExternalInput").ap()
    rotm = nc.dram_tensor("rotm", [P, P], F32R, kind="ExternalInput").ap()
    m0 = nc.dram_tensor("m0", [P, P], F32R, kind="ExternalInput").ap()
    m4 = nc.dram_tensor("m4", [P, P], F32R, kind="ExternalInput").ap()
    ones = nc.dram_tensor("ones", [P, 1], F32R, kind="ExternalInput").ap()
    ident = nc.dram_tensor("ident", [P, P], F32, kind="ExternalInput").ap()
    y = nc.dram_tensor("y", [T, D_MODEL], F32, kind="ExternalOutput").ap()

    with tile.TileContext(nc) as tc:
        with (
            tc.tile_pool(name="const", bufs=1) as cpool,
            tc.tile_pool(name="work", bufs=2) as wpool,
            tc.tile_pool(name="psum", bufs=6, space="PSUM") as pspool,
        ):
            # --- resident tensors (table DMAs ride the idle SWDGE queue and are
            # issued mid-qkv so the first matmuls aren't starved) --------------
            wq_t = cpool.tile([P, KC, DOUT], F32R, tag="wqkv")
            wo_t = cpool.tile([P, QH, D_MODEL], F32R, tag="wout")
            cos_t = cpool.tile([P, T], F32, tag="cosT")
            sin_t = cpool.tile([P, T], F32, tag="sinS")
            m0_t = cpool.tile([P, P], F32R, tag="m0")
            m4_t = cpool.tile([P, P], F32R, tag="m4")
            ones_t = cpool.tile([P, 1], F32R, tag="ones")
            rot_t = cpool.tile([P, P], F32R, tag="rotm")
            id_t = cpool.tile([P, P], F32, tag="ident")
            wq_loaded = [False] * KC
            wo_loaded = [False]
            tables_loaded = [False]

            k_res = cpool.tile([P, T], F32R, tag="k_res")   # K^T rope'd [d, tok]
            v_res = cpool.tile([P, T], F32R, tag="v_res")   # V as tok-chunks [tok, dv]

            ps_q_by_sb = {}
            q_cur_by_sb = {}
            ot_by_sb = {}

            def qkv_block(a):
                tok = slice(a * SB, (a + 1) * SB)
                ps_q = [pspool.tile([P, SB], F32, tag="ps", name=f"qkv_{a}_{m}")
                        for m in range(MQK)]
                ps_q_by_sb[a] = ps_q
                for k in range(KC):
                    if not wq_loaded[k]:
                        # ACT-issued HWDGE queue, parallel with SP's xk loads
                        nc.scalar.dma_start(wq_t[:, k, :], wqkv[k * P:(k + 1) * P, :])
                        wq_loaded[k] = True
                    xk = wpool.tile([P, SB], F32R, tag="xT", bufs=3)
                    nc.sync.dma_start(xk[:], xT[k * P:(k + 1) * P, tok])
                    for m in range(MQK):
                        nc.tensor.matmul(
                            ps_q[m][:],
                            wq_t[:, k, m * P:(m + 1) * P],
                            xk[:],
                            start=(k == 0),
                            stop=(k == KC - 1),
                        )
                    if not tables_loaded[0] and k == 1:
                        nc.gpsimd.dma_start(rot_t[:], rotm[:])
                        nc.gpsimd.dma_start(cos_t[:], cosT[:])
                        nc.gpsimd.dma_start(sin_t[:], sinS[:])
                        nc.gpsimd.dma_start(id_t[:], ident[:])
                        nc.gpsimd.dma_start(m0_t[:], m0[:])
                        nc.gpsimd.dma_start(m4_t[:], m4[:])
                        nc.gpsimd.dma_start(ones_t[:], ones[:])
                        tables_loaded[0] = True
                if not wo_loaded[0]:
                    nc.gpsimd.dma_start(wo_t[:], wout.rearrange("(c p) n -> p c n", p=P))
                    wo_loaded[0] = True

            raw_by_sb = {}

            def rope_copies(a):
                # drain the qkv psums early so outproj can reuse the slots
                ps_q = ps_q_by_sb.pop(a)
                raws = []
                for m in (4, 0, 1, 2, 3):   # K first: attention needs it earliest
                    raw = wpool.tile([P, SB], F32R, tag="rope_raw", bufs=6)
                    nc.scalar.copy(raw[:], ps_q[m][:])
                    raws.append((m, raw))
                vraw = wpool.tile([P, SB], F32, tag="vraw")
                nc.scalar.copy(vraw[:], ps_q[5][:])
                raw_by_sb[a] = (raws, vraw)

            def rope_math(a):
                tok = slice(a * SB, (a + 1) * SB)
                raws, vraw = raw_by_sb.pop(a)
                q_cur = wpool.tile([P, QH, SB], F32R, tag="q_cur", bufs=1)
                q_cur_by_sb[a] = q_cur
                for m, raw in raws:
                    rot_ps = pspool.tile([P, SB], F32, tag="ps2", bufs=2, name=f"rot_{a}_{m}")
                    nc.tensor.matmul(rot_ps[:], rot_t[:], raw[:], start=True, stop=True)
                    t1 = wpool.tile([P, SB], F32, tag="rope_t1")
                    nc.vector.tensor_mul(out=t1[:], in0=raw[:], in1=cos_t[:, tok])
                    t2 = wpool.tile([P, SB], F32, tag="rope_t2")
                    nc.vector.tensor_mul(out=t2[:], in0=rot_ps[:], in1=sin_t[:, tok])
                    dest = q_cur[:, m, :] if m < QH else k_res[:, tok]
                    nc.vector.tensor_add(out=dest, in0=t1[:], in1=t2[:])
                for t in range(SB // P):
                    ptt = pspool.tile([P, P], F32, tag="ps2", bufs=2, name=f"tr_{a}_{t}")
                    nc.tensor.transpose(ptt[:], vraw[:, t * P:(t + 1) * P], id_t[:])
                    nc.scalar.copy(v_res[:, (4 * a + t) * P:(4 * a + t + 1) * P], ptt[:])

            def att_block(a):
                q_cur = q_cur_by_sb.pop(a)
                ot_all = []
                for h in range(QH):
                    ot_ps = pspool.tile([P, SB], F32, tag="ps", name=f"ot_{a}_{h}")
                    sums = pspool.tile([1, SB], F32, tag="ps2", bufs=2, name=f"sum_{a}_{h}")
                    valid = [j for j in range(8) if 4 * a - 4 + j >= 0]
                    # j=4 spans all 512 q-columns; run it first so the PSUM
                    # bank-clearing start=True matmul covers the full bank.
                    jorder = [4] + [j for j in valid if j != 4]
                    pT_by_j = {}
                    for j in jorder:
                        ki = 4 * a - 4 + j
                        qlo = P * max(0, j - 4)
                        qhi = P * (min(3, j) + 1)
                        s_ps = pspool.tile([P, SB], F32, tag="ps", name=f"s_{a}_{h}_{j}")
                        nc.tensor.matmul(
                            s_ps[:, qlo:qhi],
                            k_res[:, ki * P:(ki + 1) * P],
                            q_cur[:, h, qlo:qhi],
                            start=True,
                            stop=True,
                        )
                        pT = wpool.tile([P, SB], F32R, tag="pT", bufs=8)
                        pT_by_j[j] = pT
                        nc.scalar.activation(
                            pT[:, qlo:qhi], s_ps[:, qlo:qhi],
                            mybir.ActivationFunctionType.Exp, scale=SCALE,
                        )
                        if j <= 3:
                            seg = slice(j * P, (j + 1) * P)
                            nc.vector.tensor_mul(out=pT[:, seg], in0=pT[:, seg], in1=m4_t[:])
                        else:
                            seg = slice((j - 4) * P, (j - 3) * P)
                            nc.vector.tensor_mul(out=pT[:, seg], in0=pT[:, seg], in1=m0_t[:])
                    for j in jorder:
                        ki = 4 * a - 4 + j
                        qlo = P * max(0, j - 4)
                        qhi = P * (min(3, j) + 1)
                        nc.tensor.matmul(
                            ot_ps[:, qlo:qhi],
                            v_res[:, ki * P:(ki + 1) * P],
                            pT_by_j[j][:, qlo:qhi],
                            start=(j == jorder[0]),
                            stop=(j == jorder[-1]),
                        )
                        # softmax denominator: accumulate column sums of pT
                        nc.tensor.matmul(
                            sums[:, qlo:qhi], ones_t[:], pT_by_j[j][:, qlo:qhi],
                            start=(j == jorder[0]), stop=(j == jorder[-1]),
                        )
                    rrow = wpool.tile([1, SB], F32, tag="rrow")
                    nc.vector.reciprocal(rrow[:], sums[:])
                    rbc = wpool.tile([P, SB], F32, tag="rbc")
                    nc.gpsimd.partition_broadcast(rbc[:], rrow[:], channels=P)
                    ot_sb = wpool.tile([P, SB], F32R, tag=f"oT{h}", bufs=1)
                    nc.vector.tensor_mul(out=ot_sb[:], in0=ot_ps[:], in1=rbc[:])
                    ot_all.append(ot_sb)
                ot_by_sb[a] = ot_all

            def out_block(a, final=False):
                ot_all = ot_by_sb.pop(a)
                for t in range(SB // P):
                    yt = wpool.tile([P, D_MODEL], F32, tag="ytile", bufs=2)
                    for n in range(D_MODEL // SB):
                        py = pspool.tile([P, SB], F32, tag="ps", name=f"y_{a}_{t}_{n}")
                        for h in range(QH):
                            nc.tensor.matmul(
                                py[:],
                                ot_all[h][:, t * P:(t + 1) * P],
                                wo_t[:, h, n * SB:(n + 1) * SB],
                                start=(h == 0),
                                stop=(h == QH - 1),
                            )
                        nc.scalar.copy(yt[:, n * SB:(n + 1) * SB], py[:])
                        if final:
                            # store per-chunk so the kernel-tail drain is short
                            eng = nc.sync if n % 2 == 0 else nc.gpsimd
                            eng.dma_start(
                                y[a * SB + t * P: a * SB + (t + 1) * P,
                                  n * SB:(n + 1) * SB],
                                yt[:, n * SB:(n + 1) * SB])
                    if not final:
                        # one row-wide store; alternate queues for the drain
                        eng = nc.sync if t % 2 == 0 else nc.gpsimd
                        eng.dma_start(
                            y[a * SB + t * P: a * SB + (t + 1) * P, :], yt[:])

            # per-sb: qkv -> rope; previous sb's outproj fills the rope drain
            # window; attention last (its exp pipeline overlaps outproj DMAs).
            for a in range(N_SB):
                qkv_block(a)
                rope_copies(a)
                if a >= 1:
                    out_block(a - 1)
                rope_math(a)
                att_block(a)
            out_block(N_SB - 1, final=True)

    nc.compile()
    return nc


def _host_tables():
    inv_freq = 1.0 / (THETA ** (np.arange(0, HEAD_DIM, 2, dtype=np.float32) / HEAD_DIM))
    pos = np.arange(T, dtype=np.float32)
    freqs = np.outer(pos, inv_freq)                     # [T, 64]
    emb = np.concatenate([freqs, freqs], axis=-1)       # [T, 128]
    cosT = np.ascontiguousarray(np.cos(emb).T.astype(np.float32))  # [128, T]
    sinS = np.ascontiguousarray(np.sin(emb).T.astype(np.float32))
    rotM = np.zeros((P, P), dtype=np.float32)
    for d in range(64):
        rotM[d, d + 64] = -1.0        # rotate-half: out[d] = -in[d+64]
        rotM[d + 64, d] = 1.0         #              out[d+64] = in[d]
    rotmat = np.ascontiguousarray(rotM.T)  # lhsT for out = rotM @ in
    kk = np.arange(P)[:, None]
    qq = np.arange(P)[None, :]
    m0 = (kk <= qq).astype(np.float32)                  # causal diag, [k, q] layout
    m4 = (kk > qq).astype(np.float32)                   # window edge
    ones = np.ones((P, 1), dtype=np.float32)
    ident = np.eye(P, dtype=np.float32)
    return cosT, sinS, rotmat, m0, m4, ones, ident


def kernel(x, Wqkv, Wout):
    x = np.asarray(x, dtype=np.float32)
    Wqkv = np.asarray(Wqkv, dtype=np.float32)
    Wout = np.asarray(Wout, dtype=np.float32)

    if "nc" not in _CACHE:
        _CACHE["nc"] = _build_program()
    nc = _CACHE["nc"]

    cosT, sinS, rotmat, m0, m4, ones, ident = _host_tables()
    xTs = [np.ascontiguousarray(x[b].T) for b in range(B)]

    in_maps = []
    for c in range(8):
        b, g = divmod(c, N_KV_HEADS)
        wq = Wqkv[:, g * QH * HEAD_DIM:(g + 1) * QH * HEAD_DIM]
        wk = Wqkv[:, D_MODEL + g * HEAD_DIM: D_MODEL + (g + 1) * HEAD_DIM]
        wv = Wqkv[:, D_MODEL + KV_DIM + g * HEAD_DIM: D_MODEL + KV_DIM + (g + 1) * HEAD_DIM]
        wqkv_sh = np.ascontiguousarray(np.concatenate([wq, wk, wv], axis=1))
        wout_sh = np.ascontiguousarray(Wout[g * QH * HEAD_DIM:(g + 1) * QH * HEAD_DIM])
        in_maps.append({
            "xT": xTs[b], "wqkv": wqkv_sh, "wout": wout_sh,
            "cosT": cosT, "sinS": sinS, "rotm": rotmat, "m0": m0, "m4": m4,
            "ones": ones, "ident": ident,
        })

    res = run_bass_kernel_spmd(nc, in_maps, core_ids=list(range(8)))

    y = np.zeros((B, T, D_MODEL), dtype=np.float32)
    for c in range(8):
        b = c // N_KV_HEADS
        y[b] += res.results[c]["y"]
    return y


# revision 22
# speedup vs baseline: 26332.6991x; 26332.6991x over previous
"""Trainium2 Bass kernel: causal self-attention with GQA + RoPE + sliding window.

Model (hardcoded from the problem spec):
  D_MODEL=2048, N_HEADS=16 (head_dim 128), N_KV_HEADS=4, T=2048, B=2,
  SLIDING_WINDOW=512, THETA=10000.

Sharding: 8 cores = batch(2) x kv-groups(4). Core (b, g) handles batch b and
query heads 4g..4g+3 with kv head g (Wqkv column-sharded). Output projection
is row-sharded (rows 512g..512g+512); the 4 partial products per batch are
summed on the host.

On-chip layout is feature-major ("transposed"): x is fed pre-transposed
(host-side) as xT [d_model, T], the QKV projection produces qkv^T
[d_out, tok], attention runs on S^T = K@Q^T tiles [k, q] so softmax
normalization uses a ones-vector matmul for the partition-dim sum, and the
PV product directly yields O^T [dv, q] which is the natural lhsT for the
output projection. All matmuls run in float32r (TF32-like, full PE rate at
moving-dim >= 256).
"""

import math

import numpy as np

try:
    import concourse.bass as bass
except ImportError:  # pragma: no cover - environment fallback
    import sys

    sys.path.insert(0, "/opt/trn_rl_repo")
    import concourse.bass as bass

import concourse.mybir as mybir
import concourse.tile as tile
from concourse import bacc
from concourse.bass_utils import run_bass_kernel_spmd

D_MODEL = 2048
N_HEADS = 16
N_KV_HEADS = 4
HEAD_DIM = 128
KV_DIM = N_KV_HEADS * HEAD_DIM  # 512
T = 2048
B = 2
SW = 512
THETA = 10000.0

P = 128
SB = 512                 # token super-block
N_SB = T // SB           # 4
KC = D_MODEL // P        # 16 contraction chunks
QH = 4                   # query heads per core
DOUT = QH * HEAD_DIM + 2 * HEAD_DIM  # 768 sharded qkv out dim
MQK = DOUT // P          # 6 dout chunks (0..3 Q, 4 K, 5 V)
SCALE = 1.0 / math.sqrt(HEAD_DIM)

F32 = mybir.dt.float32
F32R = mybir.dt.float32r

_CACHE = {}


def _build_program(repeat=1):
    nc = bacc.Bacc("TRN2", target_bir_lowering=False, debug=False, num_devices=8)

    xT = nc.dram_tensor("xT", [D_MODEL, T], F32R, kind="ExternalInput").ap()
    wqkv = nc.dram_tensor("wqkv", [D_MODEL, DOUT], F32R, kind="ExternalInput").ap()
    wout = nc.dram_tensor("wout", [QH * HEAD_DIM, D_MODEL], F32R, kind="ExternalInput").ap()
    cosT = nc.dram_tensor("cosT", [P, T], F32, kind="ExternalInput").ap()
    sinS = nc.dram_tensor("sinS", [P, T], F32, kind="ExternalInput").ap()
    rotm = nc.dram_tensor("rotm", [P, P], F32R, kind="ExternalInput").ap()
    m0 = nc.dram_tensor("m0", [P, P], F32R, kind="ExternalInput").ap()
    m4 = nc.dram_tensor("m4", [P, P], F32R, kind="ExternalInput").ap()
    ones = nc.dram_tensor("ones", [P, 1], F32R, kind="ExternalInput").ap()
    ident = nc.dram_tensor("ident", [P, P], F32, kind="ExternalInput").ap()
    y = nc.dram_tensor("y", [T, D_MODEL], F32, kind="ExternalOutput").ap()

    with tile.TileContext(nc) as tc:
        with (
            tc.tile_pool(name="const", bufs=1) as cpool,
            tc.tile_pool(name="work", bufs=2) as wpool,
            tc.tile_pool(name="psum", bufs=6, space="PSUM") as pspool,
        ):
            # --- resident tensors (table DMAs ride the idle SWDGE queue and are
            # issued mid-qkv so the first matmuls aren't starved) --------------
            wq_t = cpool.tile([P, KC, DOUT], F32R, tag="wqkv")
            wo_t = cpool.tile([P, QH, D_MODEL], F32R, tag="wout")
            cos_t = cpool.tile([P, T], F32, tag="cosT")
            sin_t = cpool.tile([P, T], F32, tag="sinS")
            m0_t = cpool.tile([P, P], F32R, tag="m0")
            m4_t = cpool.tile([P, P], F32R, tag="m4")
            ones_t = cpool.tile([P, 1], F32R, tag="ones")
            rot_t = cpool.tile([P, P], F32R, tag="rotm")
            id_t = cpool.tile([P, P], F32, tag="ident")
            wq_loaded = [False] * KC
            wo_loaded = [False]
            tables_loaded = [False]

            k_res = cpool.tile([P, T], F32R, tag="k_res")   # K^T rope'd [d, tok]
            v_res = cpool.tile([P, T], F32R, tag="v_res")   # V as tok-chunks [tok, dv]

            ps_q_by_sb = {}
            q_cur_by_sb = {}
            ot_by_sb = {}

            def qkv_block(a):
                tok = slice(a * SB, (a + 1) * SB)
                ps_q = [pspool.tile([P, SB], F32, tag="ps", name=f"qkv_{a}_{m}")
                        for m in range(MQK)]
                ps_q_by_sb[a] = ps_q
                for k in range(KC):
                    if not wq_loaded[k]:
                        # ACT-issued HWDGE queue, parallel with SP's xk loads
                        nc.scalar.dma_start(wq_t[:, k, :], wqkv[k * P:(k + 1) * P, :])
                        wq_loaded[k] = True
                    xk = wpool.tile([P, SB], F32R, tag="xT", bufs=3)
                    nc.sync.dma_start(xk[:], xT[k * P:(k + 1) * P, tok])
                    for m in range(MQK):
                        nc.tensor.matmul(
                            ps_q[m][:],
                            wq_t[:, k, m * P:(m + 1) * P],
                            xk[:],
                            start=(k == 0),
                            stop=(k == KC - 1),
                        )
                    if not tables_loaded[0] and k == 1:
                        nc.gpsimd.dma_start(rot_t[:], rotm[:])
                        nc.gpsimd.dma_start(cos_t[:], cosT[:])
                        nc.gpsimd.dma_start(sin_t[:], sinS[:])
                        nc.gpsimd.dma_start(id_t[:], ident[:])
                        nc.gpsimd.dma_start(m0_t[:], m0[:])
                        nc.gpsimd.dma_start(m4_t[:], m4[:])
                        nc.gpsimd.dma_start(ones_t[:], ones[:])
                        tables_loaded[0] = True
                if not wo_loaded[0]:
                    nc.gpsimd.dma_start(wo_t[:], wout.rearrange("(c p) n -> p c n", p=P))
                    wo_loaded[0] = True

            raw_by_sb = {}

            def rope_copies(a):
                # drain the qkv psums early so outproj can reuse the slots
                ps_q = ps_q_by_sb.pop(a)
                raws = []
                for m in (4, 0, 1, 2, 3):   # K first: attention needs it earliest
                    raw = wpool.tile([P, SB], F32R, tag="rope_raw", bufs=6)
                    nc.scalar.copy(raw[:], ps_q[m][:])
                    raws.append((m, raw))
                vraw = wpool.tile([P, SB], F32, tag="vraw")
                nc.scalar.copy(vraw[:], ps_q[5][:])
                raw_by_sb[a] = (raws, vraw)

            def rope_math(a):
                tok = slice(a * SB, (a + 1) * SB)
                raws, vraw = raw_by_sb.pop(a)
                q_cur = wpool.tile([P, QH, SB], F32R, tag="q_cur", bufs=1)
                q_cur_by_sb[a] = q_cur
                for m, raw in raws:
                    rot_ps = pspool.tile([P, SB], F32, tag="ps2", bufs=2, name=f"rot_{a}_{m}")
                    nc.tensor.matmul(rot_ps[:], rot_t[:], raw[:], start=True, stop=True)
                    t1 = wpool.tile([P, SB], F32, tag="rope_t1")
                    nc.vector.tensor_mul(out=t1[:], in0=raw[:], in1=cos_t[:, tok])
                    t2 = wpool.tile([P, SB], F32, tag="rope_t2")
                    nc.vector.tensor_mul(out=t2[:], in0=rot_ps[:], in1=sin_t[:, tok])
                    dest = q_cur[:, m, :] if m < QH else k_res[:, tok]
                    nc.vector.tensor_add(out=dest, in0=t1[:], in1=t2[:])
                for t in range(SB // P):
                    ptt = pspool.tile([P, P], F32, tag="ps2", bufs=2, name=f"tr_{a}_{t}")
                    nc.tensor.transpose(ptt[:], vraw[:, t * P:(t + 1) * P], id_t[:])
                    nc.scalar.copy(v_res[:, (4 * a + t) * P:(4 * a + t + 1) * P], ptt[:])

            def att_block(a):
                q_cur = q_cur_by_sb.pop(a)
                ot_all = []
                for h in range(QH):
                    ot_ps = pspool.tile([P, SB], F32, tag="ps", name=f"ot_{a}_{h}")
                    sums = pspool.tile([1, SB], F32, tag="ps2", bufs=2, name=f"sum_{a}_{h}")
                    valid = [j for j in range(8) if 4 * a - 4 + j >= 0]
                    # j=4 spans all 512 q-columns; run it first so the PSUM
                    # bank-clearing start=True matmul covers the full bank.
                    jorder = [4] + [j for j in valid if j != 4]
                    pT_by_j = {}
                    for j in jorder:
                        ki = 4 * a - 4 + j
                        qlo = P * max(0, j - 4)
                        qhi = P * (min(3, j) + 1)
                        s_ps = pspool.tile([P, SB], F32, tag="ps", name=f"s_{a}_{h}_{j}")
                        nc.tensor.matmul(
                            s_ps[:, qlo:qhi],
                            k_res[:, ki * P:(ki + 1) * P],
                            q_cur[:, h, qlo:qhi],
                            start=True,
                            stop=True,
                        )
                        pT = wpool.tile([P, SB], F32R, tag="pT", bufs=8)
                        pT_by_j[j] = pT
                        nc.scalar.activation(
                            pT[:, qlo:qhi], s_ps[:, qlo:qhi],
                            mybir.ActivationFunctionType.Exp, scale=SCALE,
                        )
                        if j <= 3:
                            seg = slice(j * P, (j + 1) * P)
                            nc.vector.tensor_mul(out=pT[:, seg], in0=pT[:, seg], in1=m4_t[:])
                        else:
                            seg = slice((j - 4) * P, (j - 3) * P)
                            nc.vector.tensor_mul(out=pT[:, seg], in0=pT[:, seg], in1=m0_t[:])
                    for j in jorder:
                        ki = 4 * a - 4 + j
                        qlo = P * max(0, j - 4)
                        qhi = P * (min(3, j) + 1)
                        nc.tensor.matmul(
                            ot_ps[:, qlo:qhi],
                            v_res[:, ki * P:(ki + 1) * P],
                            pT_by_j[j][:, qlo:qhi],
                            start=(j == jorder[0]),
                            stop=(j == jorder[-1]),
                        )
                        # softmax denominator: accumulate column sums of pT
                        nc.tensor.matmul(
                            sums[:, qlo:qhi], ones_t[:], pT_by_j[j][:, qlo:qhi],
                            start=(j == jorder[0]), stop=(j == jorder[-1]),
                        )
                    rrow = wpool.tile([1, SB], F32, tag="rrow")
                    nc.vector.reciprocal(rrow[:], sums[:])
                    rbc = wpool.tile([P, SB], F32, tag="rbc")
                    nc.gpsimd.partition_broadcast(rbc[:], rrow[:], channels=P)
                    ot_sb = wpool.tile([P, SB], F32R, tag=f"oT{h}", bufs=1)
                    nc.vector.tensor_mul(out=ot_sb[:], in0=ot_ps[:], in1=rbc[:])
                    ot_all.append(ot_sb)
                ot_by_sb[a] = ot_all

            def out_block(a, final=False):
                ot_all = ot_by_sb.pop(a)
                for t in range(SB // P):
                    yt = wpool.tile([P, D_MODEL], F32, tag="ytile", bufs=2)
                    for n in range(D_MODEL // SB):
                        py = pspool.tile([P, SB], F32, tag="ps", name=f"y_{a}_{t}_{n}")
                        for h in range(QH):
                            nc.tensor.matmul(
                                py[:],
                                ot_all[h][:, t * P:(t + 1) * P],
                                wo_t[:, h, n * SB:(n + 1) * SB],
                                start=(h == 0),
                                stop=(h == QH - 1),
                            )
                        nc.scalar.copy(yt[:, n * SB:(n + 1) * SB], py[:])
                        if final:
                            # store per-chunk so the kernel-tail drain is short
                            eng = nc.sync if n % 2 == 0 else nc.gpsimd
                            eng.dma_start(
                                y[a * SB + t * P: a * SB + (t + 1) * P,
                                  n * SB:(n + 1) * SB],
                                yt[:, n * SB:(n + 1) * SB])
                    if not final:
                        # one row-wide store; alternate queues for the drain
                        eng = nc.sync if t % 2 == 0 else nc.gpsimd
                        eng.dma_start(
                            y[a * SB + t * P: a * SB + (t + 1) * P, :], yt[:])

            # per-sb: qkv -> rope; previous sb's outproj fills the rope drain
            # window; attention last (its exp pipeline overlaps outproj DMAs).
            for _rep in range(repeat):
                for a in range(N_SB):
                    qkv_block(a)
                    rope_copies(a)
                    if a >= 1:
                        out_block(a - 1)
                    rope_math(a)
                    att_block(a)
                out_block(N_SB - 1, final=(_rep == repeat - 1))

    nc.compile()
    return nc


def _host_tables():
    inv_freq = 1.0 / (THETA ** (np.arange(0, HEAD_DIM, 2, dtype=np.float32) / HEAD_DIM))
    pos = np.arange(T, dtype=np.float32)
    freqs = np.outer(pos, inv_freq)                     # [T, 64]
    emb = np.concatenate([freqs, freqs], axis=-1)       # [T, 128]
    cosT = np.ascontiguousarray(np.cos(emb).T.astype(np.float32))  # [128, T]
    sinS = np.ascontiguousarray(np.sin(emb).T.astype(np.float32))
    rotM = np.zeros((P, P), dtype=np.float32)
    for d in range(64):
        rotM[d, d + 64] = -1.0        # rotate-half: out[d] = -in[d+64]
        rotM[d + 64, d] = 1.0         #              out[d+64] = in[d]
    rotmat = np.ascontiguousarray(rotM.T)  # lhsT for out = rotM @ in
    kk = np.arange(P)[:, None]
    qq = np.arange(P)[None, :]
    m0 = (kk <= qq).astype(np.float32)                  # causal diag, [k, q] layout
    m4 = (kk > qq).astype(np.float32)                   # window edge
    ones = np.ones((P, 1), dtype=np.float32)
    ident = np.eye(P, dtype=np.float32)
    return cosT, sinS, rotmat, m0, m4, ones, ident


def kernel(x, Wqkv, Wout):
    x = np.asarray(x, dtype=np.float32)
    Wqkv = np.asarray(Wqkv, dtype=np.float32)
    Wout = np.asarray(Wout, dtype=np.float32)

    if "nc" not in _CACHE:
        _CACHE["nc"] = _build_program()
    nc = _CACHE["nc"]

    cosT, sinS, rotmat, m0, m4, ones, ident = _host_tables()
    xTs = [np.ascontiguousarray(x[b].T) for b in range(B)]

    in_maps = []
    for c in range(8):
        b, g = divmod(c, N_KV_HEADS)
        wq = Wqkv[:, g * QH * HEAD_DIM:(g + 1) * QH * HEAD_DIM]
        wk = Wqkv[:, D_MODEL + g * HEAD_DIM: D_MODEL + (g + 1) * HEAD_DIM]
        wv = Wqkv[:, D_MODEL + KV_DIM + g * HEAD_DIM: D_MODEL + KV_DIM + (g + 1) * HEAD_DIM]
        wqkv_sh = np.ascontiguousarray(np.concatenate([wq, wk, wv], axis=1))
        wout_sh = np.ascontiguousarray(Wout[g * QH * HEAD_DIM:(g + 1) * QH * HEAD_DIM])
        in_maps.append({
            "xT": xTs[b], "wqkv": wqkv_sh, "wout": wout_sh,
            "cosT": cosT, "sinS": sinS, "rotm": rotmat, "m0": m0, "m4": m4,
            "ones": ones, "ident": ident,
        })

    res = run_bass_kernel_spmd(nc, in_maps, core_ids=list(range(8)))

    y = np.zeros((B, T, D_MODEL), dtype=np.float32)
    for c in range(8):
        b = c // N_KV_HEADS
        y[b] += res.results[c]["y"]
    return y
